# revision 23
# baseline (speedup 1.0000x reference)
# MoE routing + sparse-frequency inverse FFT2 kernel for Trainium2 (8 NeuronCores).
#
# Math: out_b = ALPHA * Re(ifft2(mask_b)) where mask_b has 4096 nonzero
# frequencies (top-2 experts x 2048 each).  With the symmetric real DFT basis
#   C[x,u] = cos(2*pi*x*u/768)/768,  S[x,u] = sin(2*pi*x*u/768)/768
# the dense iFFT2 factorizes per sample into  out = a*(C M C - S M S)  and the
# four-fold symmetry
#   out[x, y]     = t1 - t2          t1 = a*(C M C)[x, y<386]
#   out[x, N-y]   = t1 + t2          t2 = a*(S M S)[x, y<386]
#   out[N-x, y]   = t1 + t2
#   out[N-x, N-y] = t1 - t2
# lets stage 2 compute only 386 of 768 columns; full rows are assembled with
# negative-stride copies and rows 512..767 with an anti-identity row-mirror
# matmul.  sqrt(ALPHA) is folded into the C/S tables on the host.
# Device work per core (4 samples): router GEMM, top-2 selection and weights,
# per-expert entry gather (row-granular indirect DMA of one interleaved
# (u, v, coeff) table), sparse->dense mask build via iota/compare one-hots
# placed with PE matmuls (2 x 384-wide u-buckets per v-chunk, pad 256), then
# stage-1 P=M@C / Q=M@S at 386 columns and quadrant-assembled stage 2.
#
# Element-granular DMA scatter is avoided on purpose: TRN2's indirect DMA is
# row-granular (one offset per partition, contiguous run per partition), so
# the mask is built from gathered (u, v, val) entry groups instead.

import sys

sys.path.insert(0, "/opt/trn_rl_repo")

import numpy as np

import concourse.bacc as bacc
import concourse.mybir as mybir
import concourse.tile as tile
from concourse.bass import IndirectOffsetOnAxis
from concourse.bass_utils import run_bass_kernel_spmd
from concourse.masks import make_identity

N = 768
E = 64
NF = 2048
B = 32
NCORES = 8
BPC = B // NCORES          # samples per core
NBLK = 6                   # 768 / 128
ALPHA = 300.0
GRID = N * N
HALF = N // 2 + 2          # 386 computed stage-1/2 columns (even for f32r)
FLW = HALF - 4             # 382: width of the mirrored column range

# per-(expert, v-chunk, u-half) buckets: u in [0,384) and [384,768), each
# padded to 256 entries (exact max fill for the fixed input draw is 207).
UW = 384                   # u-bucket width (one psum bank at f32)
PAD = 256                  # entries per bucket -> 2 gather columns
BROW = 2 * PAD             # 512 entries per (expert, v-chunk)
EROW = NBLK * BROW         # 3072 entries per expert
COLS = EROW // 128         # 24 gather columns per expert
TCOLS = 3 * COLS           # 72: interleaved (u, vm, coeff) merged table

F32 = mybir.dt.float32
F32R = mybir.dt.float32r
F16 = mybir.dt.float16
I32 = mybir.dt.int32
AOT = mybir.AluOpType

KERNEL_TRACE = False       # test harness can flip this to profile
LAST_RESULT = None

_NC = None


def _build():
    nc = bacc.Bacc(trn_type="TRN2")

    cls4 = nc.dram_tensor("cls4", [N, BPC], F32, kind="ExternalInput")
    wr = nc.dram_tensor("wr", [N, E], F32, kind="ExternalInput")
    br = nc.dram_tensor("br", [E], F32, kind="ExternalInput")
    uvc = nc.dram_tensor("uvc", [E, 3 * EROW], F32, kind="ExternalInput")
    bases = nc.dram_tensor("bases", [2 * E, 1], F32, kind="ExternalInput")
    jm = nc.dram_tensor("jm", [128, 128], F32R, kind="ExternalInput")
    ct = nc.dram_tensor("ct", [N, N], F16, kind="ExternalInput")
    st = nc.dram_tensor("st", [N, N], F16, kind="ExternalInput")
    out4 = nc.dram_tensor("out4", [BPC, N, N], F32, kind="ExternalOutput")

    with tile.TileContext(nc) as tc:
        with (
            tc.tile_pool(name="const", bufs=1) as cpool,
            tc.tile_pool(name="tables", bufs=1) as tpool,
            tc.tile_pool(name="routing", bufs=1) as rpool,
            tc.tile_pool(name="gath", bufs=1) as gpool,
            tc.tile_pool(name="build", bufs=24) as bpool,
            tc.tile_pool(name="mt", bufs=2) as mtpool,
            tc.tile_pool(name="pq", bufs=2) as pqpool,
            tc.tile_pool(name="outp", bufs=6) as opool,
            tc.tile_pool(name="psH", bufs=6, space="PSUM") as psH,
            tc.tile_pool(name="psB", bufs=2, space="PSUM") as psB,
            tc.tile_pool(name="mir", bufs=2) as mirpool,
            tc.tile_pool(name="t1p", bufs=2) as t1pool,
        ):
            ident = cpool.tile([128, 128], F32)
            make_identity(nc, ident[:])
            ones1 = cpool.tile([1, 128], F32)
            nc.vector.memset(ones1[:], 1.0)
            ones14 = cpool.tile([1, BPC], F32)
            nc.vector.memset(ones14[:], 1.0)
            i768 = cpool.tile([128, N], I32)
            nc.gpsimd.iota(i768[:], pattern=[[1, N]], base=0, channel_multiplier=0)
            i768f = cpool.tile([128, N], F16)
            nc.vector.tensor_copy(i768f[:], i768[:])
            i128f = cpool.tile([128, 128], F16)
            nc.vector.tensor_copy(i128f[:], i768[:, 0:128])
            io72 = cpool.tile([128, 1], I32)
            nc.gpsimd.iota(io72[:], pattern=[[0, 1]], base=0, channel_multiplier=TCOLS)
            io72f = cpool.tile([128, 1], F32)
            nc.vector.tensor_copy(io72f[:], io72[:])

            br_sb = rpool.tile([1, E], F32)
            nc.sync.dma_start(out=br_sb[:], in_=br[None, :])
            bases_sb = rpool.tile([2 * E, 1], F32)
            nc.sync.dma_start(out=bases_sb[:], in_=bases[:])
            jJ = cpool.tile([128, 128], F32R)
            nc.sync.dma_start(out=jJ[:], in_=jm[:])

            # ---- router: logits = cls @ Wr.T + br (host pre-transposed) ----
            clsT = rpool.tile([128, NBLK * BPC], F32)
            wrT = rpool.tile([128, NBLK * E], F32)
            for j in range(NBLK):
                nc.sync.dma_start(
                    out=clsT[:, BPC * j : BPC * (j + 1)],
                    in_=cls4[128 * j : 128 * (j + 1), :],
                )
                nc.sync.dma_start(
                    out=wrT[:, E * j : E * (j + 1)],
                    in_=wr[128 * j : 128 * (j + 1), :],
                )
            lg_ps = psB.tile([BPC, E], F32, tag="small")
            for j in range(NBLK):
                nc.tensor.matmul(
                    lg_ps[:],
                    lhsT=clsT[:, BPC * j : BPC * (j + 1)],
                    rhs=wrT[:, E * j : E * (j + 1)],
                    start=(j == 0),
                    stop=False,
                )
            nc.tensor.matmul(
                lg_ps[:], lhsT=ones14[:], rhs=br_sb[:], start=False, stop=True
            )
            logits = rpool.tile([BPC, E], F32)
            nc.vector.tensor_copy(logits[:], lg_ps[:])

            # ---- top-2, renormalized weights, one-hot selectors ----
            max8 = rpool.tile([BPC, 8], F32)
            nc.vector.max(out=max8[:], in_=logits[:])
            l0 = max8[:, 0:1]
            l1 = max8[:, 1:2]
            d = rpool.tile([BPC, 1], F32)
            nc.vector.tensor_sub(d[:], l1, l0)  # l1 - l0
            dT_ps = psB.tile([1, BPC], F32, tag="small")
            nc.tensor.transpose(dT_ps[:], d[:], ident[0:BPC, 0:BPC])
            dT = rpool.tile([1, BPC], F32)
            nc.vector.tensor_copy(dT[:], dT_ps[:])
            w1T = rpool.tile([1, BPC], F32)
            nc.scalar.activation(w1T[:], dT[:], mybir.ActivationFunctionType.Sigmoid)
            w0T = rpool.tile([1, BPC], F32)
            nc.scalar.activation(
                w0T[:], dT[:], mybir.ActivationFunctionType.Sigmoid, scale=-1.0
            )
            oh1 = rpool.tile([BPC, E], F32)
            oh2 = rpool.tile([BPC, E], F32)
            nc.vector.tensor_scalar(oh1[:], logits[:], l0, None, op0=AOT.is_equal)
            nc.vector.tensor_scalar(oh2[:], logits[:], l1, None, op0=AOT.is_equal)
            selT = []
            for srcap in (oh1, oh2):
                sp = psB.tile([E, BPC], F32, tag="small")
                nc.tensor.transpose(sp[:], srcap[:], ident[0:BPC, 0:BPC])
                sbt = rpool.tile([E, BPC], F32, tag=f"selT{len(selT)}")
                nc.vector.tensor_copy(sbt[:], sp[:])
                selT.append(sbt)
            o1T, o2T = selT

            # scalar rows packed [eT0 | eT1 | w0 | w1], broadcast in one matmul
            stack = rpool.tile([1, 4 * BPC], F32)
            for si, oT in enumerate((o1T, o2T)):
                ep = psB.tile([1, BPC], F32, tag="small")
                nc.tensor.matmul(
                    ep[:], lhsT=bases_sb[0:E, :], rhs=oT[:], start=True, stop=True
                )
                nc.vector.tensor_copy(stack[:, BPC * si : BPC * (si + 1)], ep[:])
            nc.scalar.copy(stack[:, 2 * BPC : 3 * BPC], w0T[:])
            nc.scalar.copy(stack[:, 3 * BPC : 4 * BPC], w1T[:])
            bp = psB.tile([128, 4 * BPC], F32, tag="small")
            nc.tensor.matmul(bp[:], lhsT=ones1[:], rhs=stack[:], start=True, stop=True)
            bcT = rpool.tile([128, 4 * BPC], F32)
            nc.vector.tensor_copy(bcT[:], bp[:])
            ebc = [bcT[:, 0:BPC], bcT[:, BPC : 2 * BPC]]
            wbc = [bcT[:, 2 * BPC : 3 * BPC], bcT[:, 3 * BPC : 4 * BPC]]

            # ---- C/S table loads AFTER the routing-phase emission so the
            # small router DMAs aren't queued behind 4.7 MB on the sync FIFO
            ct_sb = tpool.tile([128, NBLK * N], F16, tag="ct")
            st_sb = tpool.tile([128, NBLK * N], F16, tag="st")
            for j in range(NBLK):
                nc.sync.dma_start(
                    out=ct_sb[:, N * j : N * (j + 1)],
                    in_=ct[128 * j : 128 * (j + 1), :],
                )
                nc.sync.dma_start(
                    out=st_sb[:, N * j : N * (j + 1)],
                    in_=st[128 * j : 128 * (j + 1), :],
                )

            dma_engines = (nc.sync, nc.scalar)

            # ---- gather ALL samples' interleaved (u, vm, coeff) runs upfront
            # so later samples' indirect dispatches aren't head-of-line
            # blocked on the gpsimd queue behind dependent work.
            off_tiles = {}
            for b in range(BPC):
                for slot in range(2):
                    offf = gpool.tile(
                        [128, 1], F32, tag=f"offf{b}_{slot}", name=f"offf{b}_{slot}"
                    )
                    nc.vector.tensor_add(offf[:], ebc[slot][:, b : b + 1], io72f[:])
                    offs = gpool.tile(
                        [128, 1], I32, tag=f"offs{b}_{slot}", name=f"offs{b}_{slot}"
                    )
                    nc.vector.tensor_copy(offs[:], offf[:])
                    off_tiles[(b, slot)] = offs
            allg = []
            for b in range(BPC):
                gms = []
                for slot in range(2):
                    gm = gpool.tile([128, TCOLS], F32, tag=f"gm{b}_{slot}")
                    nc.gpsimd.indirect_dma_start(
                        out=gm[:],
                        out_offset=None,
                        in_=uvc[:],
                        in_offset=IndirectOffsetOnAxis(
                            ap=off_tiles[(b, slot)][:], axis=1
                        ),
                    )
                    gms.append(gm)
                allg.append((gms, b))

            # ---- build MT (transposed mask) chunk by chunk on PE ----
            # per (j, u-half): one single-bank psum tile accumulating
            # 2 slots x 2 groups of one-hot placement matmuls.  Build of
            # sample b+1 is emitted interleaved into stage 1 of sample b so
            # the DVE one-hot production overlaps PE-dense stage matmuls.
            mt_tiles = {}

            gcw_tiles = {}

            def emit_build_chunk(b, j):
                gms, _ = allg[b]
                if j == 0:
                    mt_tiles[b] = mtpool.tile(
                        [128, NBLK * N], F16, tag="mt", name=f"mt{b}"
                    )
                    gcws = []
                    for slot in range(2):
                        gcw = gpool.tile(
                            [128, COLS], F32, tag=f"gcw{b}_{slot}",
                            name=f"gcw{b}_{slot}",
                        )
                        nc.vector.tensor_scalar(
                            gcw[:], gms[slot][:][:, 2:TCOLS:3],
                            wbc[slot][:, b : b + 1], None, op0=AOT.mult,
                        )
                        gcws.append(gcw)
                    gcw_tiles[b] = gcws
                mt_sb = mt_tiles[b]
                gcws = gcw_tiles[b]
                pss = [
                    psH.tile([128, 448], F32, tag="ps", name=f"bld{half}")
                    for half in range(2)
                ]
                for slot in range(2):
                    for g in range(2):
                        for half in range(2):
                            u0 = UW * half
                            c = 4 * j + 2 * half + g
                            voh = bpool.tile([128, 128], F16, tag="voh")
                            nc.vector.tensor_scalar(
                                voh[:], i128f[:],
                                gms[slot][:, 3 * c + 1 : 3 * c + 2],
                                None, op0=AOT.is_equal,
                            )
                            rhsb = bpool.tile([128, UW], F16, tag="rhsb")
                            nc.vector.tensor_scalar(
                                rhsb[:], i768f[:, u0 : u0 + UW],
                                gms[slot][:, 3 * c : 3 * c + 1],
                                gcws[slot][:, c : c + 1],
                                op0=AOT.is_equal, op1=AOT.mult,
                            )
                            nc.tensor.matmul(
                                pss[half][:, 0:UW],
                                lhsT=voh[:],
                                rhs=rhsb[:],
                                start=(slot == 0 and g == 0),
                                stop=(slot == 1 and g == 1),
                            )
                for half in range(2):
                    u0 = UW * half
                    nc.scalar.copy(
                        mt_sb[:, N * j + u0 : N * j + u0 + UW],
                        pss[half][:, 0:UW],
                    )

            for j in range(NBLK):
                emit_build_chunk(0, j)

            for b in range(BPC):
                mt_sb = mt_tiles[b]
                # ---- stage 1: P = M @ (sa*C), Q = M @ (sa*S), 386 cols;
                # weave next sample's build chunks between the i-blocks.
                p_sb = pqpool.tile([128, NBLK * HALF], F16, tag="p")
                q_sb = pqpool.tile([128, NBLK * HALF], F16, tag="q")
                for i in range(NBLK):
                    pps = psH.tile([128, 448], F32, tag="ps")
                    qps = psH.tile([128, 448], F32, tag="ps")
                    for k in range(NBLK):
                        lhs = mt_sb[:, N * k + 128 * i : N * k + 128 * (i + 1)]
                        nc.tensor.matmul(
                            pps[:, 0:HALF], lhsT=lhs,
                            rhs=ct_sb[:, N * k : N * k + HALF],
                            start=(k == 0), stop=(k == NBLK - 1),
                        )
                        nc.tensor.matmul(
                            qps[:, 0:HALF], lhsT=lhs,
                            rhs=st_sb[:, N * k : N * k + HALF],
                            start=(k == 0), stop=(k == NBLK - 1),
                        )
                    nc.scalar.copy(p_sb[:, HALF * i : HALF * (i + 1)], pps[:, 0:HALF])
                    nc.scalar.copy(q_sb[:, HALF * i : HALF * (i + 1)], qps[:, 0:HALF])
                    if b + 1 < BPC:
                        emit_build_chunk(b + 1, i)

                # ---- stage 2 (quadrant): t1 = a*CMC, t2 = a*SMS, cols < 386;
                # out rows 0..511 direct, 512..767 via row-mirror matmul.
                def emit_mir(di, mc, dmae):
                    d0 = psH.tile([128, 448], F32, tag="ps")
                    d1 = psH.tile([128, 448], F32, tag="ps")
                    nc.tensor.matmul(
                        d0[:, 0:448], lhsT=jJ[:], rhs=mc[:, 0:448],
                        start=True, stop=True,
                    )
                    nc.tensor.matmul(
                        d1[:, 0:320], lhsT=jJ[:], rhs=mc[:, 448:768],
                        start=True, stop=True,
                    )
                    ob = opool.tile([128, N], F32, tag="ob")
                    nc.scalar.copy(ob[:, 0:448], d0[:, 0:448])
                    nc.scalar.copy(ob[:, 448:768], d1[:, 0:320])
                    dmae.dma_start(
                        out=out4[:][b][128 * (4 + di) : 128 * (5 + di), :], in_=ob[:]
                    )

                mirs = {}
                for i in range(4):
                    t1 = psH.tile([128, 448], F32, tag="ps")
                    t2 = psH.tile([128, 448], F32, tag="ps")
                    # mirror blocks are emitted one i-block late so their PE
                    # matmuls queue behind this block's chains and never wait
                    # on the previous block's assembly ops
                    pend_mir = None
                    if i == 2:
                        pend_mir = (1, mirs[0])
                    elif i == 3:
                        pend_mir = (0, mirs[1])
                    for k in range(NBLK):
                        nc.tensor.matmul(
                            t1[:, 0:HALF],
                            lhsT=ct_sb[:, N * k + 128 * i : N * k + 128 * (i + 1)],
                            rhs=p_sb[:, HALF * k : HALF * (k + 1)],
                            start=(k == 0), stop=(k == NBLK - 1),
                        )
                        nc.tensor.matmul(
                            t2[:, 0:HALF],
                            lhsT=st_sb[:, N * k + 128 * i : N * k + 128 * (i + 1)],
                            rhs=q_sb[:, HALF * k : HALF * (k + 1)],
                            start=(k == 0), stop=(k == NBLK - 1),
                        )
                    if pend_mir is not None:
                        emit_mir(pend_mir[0], pend_mir[1], dma_engines[i % 2])
                    t1s = t1pool.tile([128, HALF], F32, tag="t1s")
                    nc.scalar.copy(t1s[:], t1[:, 0:HALF])
                    ob = opool.tile([128, N], F32, tag="ob")
                    # direct region: out = t1 - t2
                    nc.vector.tensor_tensor(
                        ob[:, 0:HALF], t1s[:], t2[:, 0:HALF], op=AOT.subtract
                    )
                    # mirror content: t1 + t2
                    if i < 2:
                        mc = mirpool.tile([128, N], F32R, tag=f"mc{i}")
                        nc.vector.tensor_tensor(
                            mc[:, 0:HALF], t1s[:], t2[:, 0:HALF], op=AOT.add
                        )
                        dsrc = mc
                        mirs[i] = mc
                    else:
                        dsrc = t1pool.tile([128, HALF], F32, tag="dtmp")
                        nc.vector.tensor_tensor(
                            dsrc[:, 0:HALF], t1s[:], t2[:, 0:HALF], op=AOT.add
                        )
                    # flipped column halves
                    nc.scalar.copy(ob[:, HALF : N], dsrc[:][:, FLW:0:-1])
                    if i < 2:
                        nc.scalar.copy(mc[:, HALF : N], ob[:][:, FLW:0:-1])
                    # row-0 patches: mirror tiles take the next block's row 0
                    if i == 1:
                        nc.vector.tensor_tensor(
                            mirs[0][0:1, 0:HALF], t1s[0:1, :], t2[0:1, 0:HALF],
                            op=AOT.add,
                        )
                        nc.scalar.copy(
                            mirs[0][0:1, HALF : N], ob[:][0:1, FLW:0:-1]
                        )
                    elif i == 2:
                        nc.vector.tensor_tensor(
                            mirs[1][0:1, 0:HALF], t1s[0:1, :], t2[0:1, 0:HALF],
                            op=AOT.add,
                        )
                        nc.scalar.copy(
                            mirs[1][0:1, HALF : N], ob[:][0:1, FLW:0:-1]
                        )
                    dma_engines[i % 2].dma_start(
                        out=out4[:][b][128 * i : 128 * (i + 1), :], in_=ob[:]
                    )

    nc.compile()
    return nc


def _get_nc():
    global _NC
    if _NC is None:
        _NC = _build()
    return _NC


def _host_tables():
    a = np.arange(N, dtype=np.int64)
    ang = (2.0 * np.pi / N) * ((a[:, None] * a[None, :]) % N)
    sa = np.sqrt(ALPHA)
    ctv = (np.cos(ang) / N * sa).astype(np.float16)
    stv = (np.sin(ang) / N * sa).astype(np.float16)
    return ctv, stv


def _host_entry_tables(list_indices, coeff):
    """Bucket each expert's (u, v, coeff) entries by (v-chunk, u-half), pad
    buckets to PAD, and interleave the three tables partition-major so one
    row-granular gather of 72 floats per partition fetches everything."""
    li = list_indices.astype(np.int64)
    uu = li // N
    vv = li % N
    u3 = np.zeros((E, EROW), np.float32)
    vm3 = np.full((E, EROW), -9.0, np.float32)
    cv3 = np.zeros((E, EROW), np.float32)
    for e in range(E):
        for j in range(NBLK):
            selj = vv[e] // 128 == j
            base = BROW * j
            for half in range(2):
                u0 = UW * half
                sel = np.where(selj & (uu[e] >= u0) & (uu[e] < u0 + UW))[0]
                cnt = len(sel)
                assert cnt <= PAD, f"bucket overflow: e{e} j{j} h{half}: {cnt}"
                u3[e, base : base + cnt] = uu[e, sel]
                vm3[e, base : base + cnt] = vv[e, sel] - 128 * j
                cv3[e, base : base + cnt] = coeff[e, sel]
                base += PAD
    # partition-major interleave: uvc[e, p*72 + 3g + t] = tab_t[e, 128*g + p]
    s = np.arange(EROW)
    g, p = s // 128, s % 128
    idx = p * TCOLS + 3 * g
    uvcv = np.zeros((E, 3 * EROW), np.float32)
    uvcv[:, idx] = u3[:, s]
    uvcv[:, idx + 1] = vm3[:, s]
    uvcv[:, idx + 2] = cv3[:, s]
    return uvcv


def kernel(cls_token, W_router, b_router, coeff, list_indices):
    global LAST_RESULT
    cls_token = np.asarray(cls_token)
    W_router = np.asarray(W_router)
    b_router = np.asarray(b_router)
    coeff = np.asarray(coeff)
    list_indices = np.asarray(list_indices)
    assert cls_token.shape == (B, N) and coeff.shape == (E, NF)
    nc = _get_nc()
    ctv, stv = _host_tables()
    uvcv = _host_entry_tables(list_indices, coeff)
    basesv = np.tile(
        (np.arange(E, dtype=np.float32) * (3 * EROW)).reshape(E, 1), (2, 1)
    )
    jmv = np.zeros((128, 128), np.float32)
    for m_ in range(128):
        jmv[(128 - m_) % 128, m_] = 1.0
    wrr = np.ascontiguousarray(W_router.T, dtype=np.float32)
    brr = np.ascontiguousarray(b_router, dtype=np.float32)
    in_maps = []
    for c in range(NCORES):
        in_maps.append(
            {
                "cls4": np.ascontiguousarray(
                    cls_token[BPC * c : BPC * (c + 1)].T, dtype=np.float32
                ),
                "wr": wrr,
                "br": brr,
                "uvc": uvcv,
                "bases": basesv,
                "jm": jmv,
                "ct": ctv,
                "st": stv,
            }
        )
    res = run_bass_kernel_spmd(
        nc, in_maps, core_ids=list(range(NCORES)), trace=KERNEL_TRACE
    )
    LAST_RESULT = res
    out = np.concatenate([res.results[c]["out4"] for c in range(NCORES)], axis=0)
    return out


# revision 24
# speedup vs baseline: 1.0106x; 1.0106x over previous
# MoE routing + sparse-frequency inverse FFT2 kernel for Trainium2 (8 NeuronCores).
#
# Math: out_b = ALPHA * Re(ifft2(mask_b)) where mask_b has 4096 nonzero
# frequencies (top-2 experts x 2048 each).  With the symmetric real DFT basis
#   C[x,u] = cos(2*pi*x*u/768)/768,  S[x,u] = sin(2*pi*x*u/768)/768
# the dense iFFT2 factorizes per sample into  out = a*(C M C - S M S)  and the
# four-fold symmetry
#   out[x, y]     = t1 - t2          t1 = a*(C M C)[x, y<386]
#   out[x, N-y]   = t1 + t2          t2 = a*(S M S)[x, y<386]
#   out[N-x, y]   = t1 + t2
#   out[N-x, N-y] = t1 - t2
# lets stage 2 compute only 386 of 768 columns; full rows are assembled with
# negative-stride copies and rows 512..767 with an anti-identity row-mirror
# matmul.  sqrt(ALPHA) is folded into the C/S tables on the host.
# Device work per core (4 samples): router GEMM, top-2 selection and weights,
# per-expert entry gather (row-granular indirect DMA of one interleaved
# (u, v, coeff) table), sparse->dense mask build via iota/compare one-hots
# placed with PE matmuls (2 x 384-wide u-buckets per v-chunk, pad 256), then
# stage-1 P=M@C / Q=M@S at 386 columns and quadrant-assembled stage 2.
#
# Element-granular DMA scatter is avoided on purpose: TRN2's indirect DMA is
# row-granular (one offset per partition, contiguous run per partition), so
# the mask is built from gathered (u, v, val) entry groups instead.

import sys

sys.path.insert(0, "/opt/trn_rl_repo")

import numpy as np

import concourse.bacc as bacc
import concourse.mybir as mybir
import concourse.tile as tile
from concourse.bass import IndirectOffsetOnAxis
from concourse.bass_utils import run_bass_kernel_spmd
from concourse.masks import make_identity

N = 768
E = 64
NF = 2048
B = 32
NCORES = 8
BPC = B // NCORES          # samples per core
NBLK = 6                   # 768 / 128
ALPHA = 300.0
GRID = N * N
HALF = N // 2 + 2          # 386 computed stage-1/2 columns (even for f32r)
FLW = HALF - 4             # 382: width of the mirrored column range

# per-(expert, v-chunk, u-half) buckets: u in [0,384) and [384,768), each
# padded to 256 entries (exact max fill for the fixed input draw is 207).
UW = 384                   # u-bucket width (one psum bank at f32)
PAD = 256                  # entries per bucket -> 2 gather columns
BROW = 2 * PAD             # 512 entries per (expert, v-chunk)
EROW = NBLK * BROW         # 3072 entries per expert
COLS = EROW // 128         # 24 gather columns per expert
TCOLS = 3 * COLS           # 72: interleaved (u, vm, coeff) merged table

F32 = mybir.dt.float32
F32R = mybir.dt.float32r
F16 = mybir.dt.float16
I32 = mybir.dt.int32
AOT = mybir.AluOpType

KERNEL_TRACE = False       # test harness can flip this to profile
LAST_RESULT = None

_NC = None


def _build():
    nc = bacc.Bacc(trn_type="TRN2")

    cls4 = nc.dram_tensor("cls4", [N, BPC], F32, kind="ExternalInput")
    wr = nc.dram_tensor("wr", [N, E], F32, kind="ExternalInput")
    br = nc.dram_tensor("br", [E], F32, kind="ExternalInput")
    uvc = nc.dram_tensor("uvc", [E, 3 * EROW], F32, kind="ExternalInput")
    bases = nc.dram_tensor("bases", [2 * E, 1], F32, kind="ExternalInput")
    jm = nc.dram_tensor("jm", [128, 128], F32R, kind="ExternalInput")
    ct = nc.dram_tensor("ct", [N, N], F16, kind="ExternalInput")
    st = nc.dram_tensor("st", [N, N], F16, kind="ExternalInput")
    out4 = nc.dram_tensor("out4", [BPC, N, N], F32, kind="ExternalOutput")

    with tile.TileContext(nc) as tc:
        with (
            tc.tile_pool(name="const", bufs=1) as cpool,
            tc.tile_pool(name="tables", bufs=1) as tpool,
            tc.tile_pool(name="routing", bufs=1) as rpool,
            tc.tile_pool(name="gath", bufs=1) as gpool,
            tc.tile_pool(name="build", bufs=24) as bpool,
            tc.tile_pool(name="mt", bufs=2) as mtpool,
            tc.tile_pool(name="pq", bufs=2) as pqpool,
            tc.tile_pool(name="outp", bufs=6) as opool,
            tc.tile_pool(name="psH", bufs=6, space="PSUM") as psH,
            tc.tile_pool(name="psB", bufs=2, space="PSUM") as psB,
            tc.tile_pool(name="mir", bufs=2) as mirpool,
            tc.tile_pool(name="t1p", bufs=2) as t1pool,
        ):
            ident = cpool.tile([128, 128], F32)
            make_identity(nc, ident[:])
            ones1 = cpool.tile([1, 128], F32)
            nc.vector.memset(ones1[:], 1.0)
            ones14 = cpool.tile([1, BPC], F32)
            nc.vector.memset(ones14[:], 1.0)
            i768 = cpool.tile([128, N], I32)
            nc.gpsimd.iota(i768[:], pattern=[[1, N]], base=0, channel_multiplier=0)
            i768f = cpool.tile([128, N], F16)
            nc.vector.tensor_copy(i768f[:], i768[:])
            i128f = cpool.tile([128, 128], F16)
            nc.vector.tensor_copy(i128f[:], i768[:, 0:128])
            io72 = cpool.tile([128, 1], I32)
            nc.gpsimd.iota(io72[:], pattern=[[0, 1]], base=0, channel_multiplier=TCOLS)
            io72f = cpool.tile([128, 1], F32)
            nc.vector.tensor_copy(io72f[:], io72[:])

            br_sb = rpool.tile([1, E], F32)
            nc.sync.dma_start(out=br_sb[:], in_=br[None, :])
            bases_sb = rpool.tile([2 * E, 1], F32)
            nc.sync.dma_start(out=bases_sb[:], in_=bases[:])
            jJ = cpool.tile([128, 128], F32R)
            nc.sync.dma_start(out=jJ[:], in_=jm[:])

            # ---- router: logits = cls @ Wr.T + br (host pre-transposed) ----
            clsT = rpool.tile([128, NBLK * BPC], F32)
            wrT = rpool.tile([128, NBLK * E], F32)
            for j in range(NBLK):
                nc.sync.dma_start(
                    out=clsT[:, BPC * j : BPC * (j + 1)],
                    in_=cls4[128 * j : 128 * (j + 1), :],
                )
                nc.sync.dma_start(
                    out=wrT[:, E * j : E * (j + 1)],
                    in_=wr[128 * j : 128 * (j + 1), :],
                )
            lg_ps = psB.tile([BPC, E], F32, tag="small")
            for j in range(NBLK):
                nc.tensor.matmul(
                    lg_ps[:],
                    lhsT=clsT[:, BPC * j : BPC * (j + 1)],
                    rhs=wrT[:, E * j : E * (j + 1)],
                    start=(j == 0),
                    stop=False,
                )
            nc.tensor.matmul(
                lg_ps[:], lhsT=ones14[:], rhs=br_sb[:], start=False, stop=True
            )
            logits = rpool.tile([BPC, E], F32)
            nc.vector.tensor_copy(logits[:], lg_ps[:])

            # ---- top-2, renormalized weights, one-hot selectors ----
            max8 = rpool.tile([BPC, 8], F32)
            nc.vector.max(out=max8[:], in_=logits[:])
            l0 = max8[:, 0:1]
            l1 = max8[:, 1:2]
            d = rpool.tile([BPC, 1], F32)
            nc.vector.tensor_sub(d[:], l1, l0)  # l1 - l0
            dT_ps = psB.tile([1, BPC], F32, tag="small")
            nc.tensor.transpose(dT_ps[:], d[:], ident[0:BPC, 0:BPC])
            dT = rpool.tile([1, BPC], F32)
            nc.vector.tensor_copy(dT[:], dT_ps[:])
            w1T = rpool.tile([1, BPC], F32)
            nc.scalar.activation(w1T[:], dT[:], mybir.ActivationFunctionType.Sigmoid)
            w0T = rpool.tile([1, BPC], F32)
            nc.scalar.activation(
                w0T[:], dT[:], mybir.ActivationFunctionType.Sigmoid, scale=-1.0
            )
            oh1 = rpool.tile([BPC, E], F32)
            oh2 = rpool.tile([BPC, E], F32)
            nc.vector.tensor_scalar(oh1[:], logits[:], l0, None, op0=AOT.is_equal)
            nc.vector.tensor_scalar(oh2[:], logits[:], l1, None, op0=AOT.is_equal)
            selT = []
            for srcap in (oh1, oh2):
                sp = psB.tile([E, BPC], F32, tag="small")
                nc.tensor.transpose(sp[:], srcap[:], ident[0:BPC, 0:BPC])
                sbt = rpool.tile([E, BPC], F32, tag=f"selT{len(selT)}")
                nc.vector.tensor_copy(sbt[:], sp[:])
                selT.append(sbt)
            o1T, o2T = selT

            # scalar rows packed [eT0 | eT1 | w0 | w1], broadcast in one matmul
            stack = rpool.tile([1, 4 * BPC], F32)
            for si, oT in enumerate((o1T, o2T)):
                ep = psB.tile([1, BPC], F32, tag="small")
                nc.tensor.matmul(
                    ep[:], lhsT=bases_sb[0:E, :], rhs=oT[:], start=True, stop=True
                )
                nc.vector.tensor_copy(stack[:, BPC * si : BPC * (si + 1)], ep[:])
            nc.scalar.copy(stack[:, 2 * BPC : 3 * BPC], w0T[:])
            nc.scalar.copy(stack[:, 3 * BPC : 4 * BPC], w1T[:])
            bp = psB.tile([128, 4 * BPC], F32, tag="small")
            nc.tensor.matmul(bp[:], lhsT=ones1[:], rhs=stack[:], start=True, stop=True)
            bcT = rpool.tile([128, 4 * BPC], F32)
            nc.vector.tensor_copy(bcT[:], bp[:])
            ebc = [bcT[:, 0:BPC], bcT[:, BPC : 2 * BPC]]
            wbc = [bcT[:, 2 * BPC : 3 * BPC], bcT[:, 3 * BPC : 4 * BPC]]

            # ---- C/S table loads AFTER the routing-phase emission so the
            # small router DMAs aren't queued behind 4.7 MB on the sync FIFO
            ct_sb = tpool.tile([128, NBLK * N], F16, tag="ct")
            st_sb = tpool.tile([128, NBLK * N], F16, tag="st")
            for j in range(NBLK):
                nc.sync.dma_start(
                    out=ct_sb[:, N * j : N * (j + 1)],
                    in_=ct[128 * j : 128 * (j + 1), :],
                )
                nc.sync.dma_start(
                    out=st_sb[:, N * j : N * (j + 1)],
                    in_=st[128 * j : 128 * (j + 1), :],
                )

            dma_engines = (nc.sync, nc.scalar)

            # ---- gather ALL samples' interleaved (u, vm, coeff) runs upfront
            # so later samples' indirect dispatches aren't head-of-line
            # blocked on the gpsimd queue behind dependent work.
            off_tiles = {}
            for b in range(BPC):
                for slot in range(2):
                    offf = gpool.tile(
                        [128, 1], F32, tag=f"offf{b}_{slot}", name=f"offf{b}_{slot}"
                    )
                    nc.vector.tensor_add(offf[:], ebc[slot][:, b : b + 1], io72f[:])
                    offs = gpool.tile(
                        [128, 1], I32, tag=f"offs{b}_{slot}", name=f"offs{b}_{slot}"
                    )
                    nc.vector.tensor_copy(offs[:], offf[:])
                    off_tiles[(b, slot)] = offs
            allg = []
            for b in range(BPC):
                gms = []
                for slot in range(2):
                    gm = gpool.tile([128, TCOLS], F32, tag=f"gm{b}_{slot}")
                    nc.gpsimd.indirect_dma_start(
                        out=gm[:],
                        out_offset=None,
                        in_=uvc[:],
                        in_offset=IndirectOffsetOnAxis(
                            ap=off_tiles[(b, slot)][:], axis=1
                        ),
                    )
                    gms.append(gm)
                allg.append((gms, b))

            # ---- build MT (transposed mask) chunk by chunk on PE ----
            # per (j, u-half): one single-bank psum tile accumulating
            # 2 slots x 2 groups of one-hot placement matmuls.  Build of
            # sample b+1 is emitted interleaved into stage 1 of sample b so
            # the DVE one-hot production overlaps PE-dense stage matmuls.
            mt_tiles = {}

            gcw_tiles = {}

            def emit_build_chunk(b, j):
                gms, _ = allg[b]
                if j == 0:
                    mt_tiles[b] = mtpool.tile(
                        [128, NBLK * N], F16, tag="mt", name=f"mt{b}"
                    )
                    gcws = []
                    for slot in range(2):
                        gcw = gpool.tile(
                            [128, COLS], F32, tag=f"gcw{b}_{slot}",
                            name=f"gcw{b}_{slot}",
                        )
                        nc.vector.tensor_scalar(
                            gcw[:], gms[slot][:][:, 2:TCOLS:3],
                            wbc[slot][:, b : b + 1], None, op0=AOT.mult,
                        )
                        gcws.append(gcw)
                    gcw_tiles[b] = gcws
                mt_sb = mt_tiles[b]
                gcws = gcw_tiles[b]
                pss = [
                    psH.tile([128, 448], F32, tag="ps", name=f"bld{half}")
                    for half in range(2)
                ]
                for slot in range(2):
                    for g in range(2):
                        for half in range(2):
                            u0 = UW * half
                            c = 4 * j + 2 * half + g
                            voh = bpool.tile([128, 128], F16, tag="voh")
                            nc.vector.tensor_scalar(
                                voh[:], i128f[:],
                                gms[slot][:, 3 * c + 1 : 3 * c + 2],
                                None, op0=AOT.is_equal,
                            )
                            rhsb = bpool.tile([128, UW], F16, tag="rhsb")
                            nc.vector.tensor_scalar(
                                rhsb[:], i768f[:, u0 : u0 + UW],
                                gms[slot][:, 3 * c : 3 * c + 1],
                                gcws[slot][:, c : c + 1],
                                op0=AOT.is_equal, op1=AOT.mult,
                            )
                            nc.tensor.matmul(
                                pss[half][:, 0:UW],
                                lhsT=voh[:],
                                rhs=rhsb[:],
                                start=(slot == 0 and g == 0),
                                stop=(slot == 1 and g == 1),
                            )
                for half in range(2):
                    u0 = UW * half
                    nc.scalar.copy(
                        mt_sb[:, N * j + u0 : N * j + u0 + UW],
                        pss[half][:, 0:UW],
                    )

            for j in range(NBLK):
                emit_build_chunk(0, j)

            for b in range(BPC):
                mt_sb = mt_tiles[b]
                # ---- stage 1: P = M @ (sa*C), Q = M @ (sa*S), 386 cols;
                # weave next sample's build chunks between the i-blocks.
                p_sb = pqpool.tile([128, NBLK * HALF], F16, tag="p")
                q_sb = pqpool.tile([128, NBLK * HALF], F16, tag="q")
                for i in range(NBLK):
                    pps = psH.tile([128, 448], F32, tag="ps")
                    qps = psH.tile([128, 448], F32, tag="ps")
                    for k in range(NBLK):
                        lhs = mt_sb[:, N * k + 128 * i : N * k + 128 * (i + 1)]
                        nc.tensor.matmul(
                            pps[:, 0:HALF], lhsT=lhs,
                            rhs=ct_sb[:, N * k : N * k + HALF],
                            start=(k == 0), stop=(k == NBLK - 1),
                        )
                        nc.tensor.matmul(
                            qps[:, 0:HALF], lhsT=lhs,
                            rhs=st_sb[:, N * k : N * k + HALF],
                            start=(k == 0), stop=(k == NBLK - 1),
                        )
                    nc.scalar.copy(p_sb[:, HALF * i : HALF * (i + 1)], pps[:, 0:HALF])
                    nc.scalar.copy(q_sb[:, HALF * i : HALF * (i + 1)], qps[:, 0:HALF])
                    if b + 1 < BPC and i < 5:
                        emit_build_chunk(b + 1, i)

                # ---- stage 2 (quadrant): t1 = a*CMC, t2 = a*SMS, cols < 386;
                # out rows 0..511 direct, 512..767 via row-mirror matmul.
                def emit_mir(di, mc, dmae):
                    d0 = psH.tile([128, 448], F32, tag="ps")
                    d1 = psH.tile([128, 448], F32, tag="ps")
                    nc.tensor.matmul(
                        d0[:, 0:448], lhsT=jJ[:], rhs=mc[:, 0:448],
                        start=True, stop=True,
                    )
                    nc.tensor.matmul(
                        d1[:, 0:320], lhsT=jJ[:], rhs=mc[:, 448:768],
                        start=True, stop=True,
                    )
                    ob = opool.tile([128, N], F32, tag="ob")
                    nc.scalar.copy(ob[:, 0:448], d0[:, 0:448])
                    nc.scalar.copy(ob[:, 448:768], d1[:, 0:320])
                    dmae.dma_start(
                        out=out4[:][b][128 * (4 + di) : 128 * (5 + di), :], in_=ob[:]
                    )

                mirs = {}
                for i in range(4):
                    t1 = psH.tile([128, 448], F32, tag="ps")
                    t2 = psH.tile([128, 448], F32, tag="ps")
                    # mirror blocks are emitted one i-block late so their PE
                    # matmuls queue behind this block's chains and never wait
                    # on the previous block's assembly ops
                    pend_mir = None
                    if i == 2:
                        pend_mir = (1, mirs[0])
                    elif i == 3:
                        pend_mir = (0, mirs[1])
                    for k in range(NBLK):
                        nc.tensor.matmul(
                            t1[:, 0:HALF],
                            lhsT=ct_sb[:, N * k + 128 * i : N * k + 128 * (i + 1)],
                            rhs=p_sb[:, HALF * k : HALF * (k + 1)],
                            start=(k == 0), stop=(k == NBLK - 1),
                        )
                        nc.tensor.matmul(
                            t2[:, 0:HALF],
                            lhsT=st_sb[:, N * k + 128 * i : N * k + 128 * (i + 1)],
                            rhs=q_sb[:, HALF * k : HALF * (k + 1)],
                            start=(k == 0), stop=(k == NBLK - 1),
                        )
                    if pend_mir is not None:
                        emit_mir(pend_mir[0], pend_mir[1], dma_engines[i % 2])
                    t1s = t1pool.tile([128, HALF], F32, tag="t1s")
                    nc.scalar.copy(t1s[:], t1[:, 0:HALF])
                    ob = opool.tile([128, N], F32, tag="ob")
                    # direct region: out = t1 - t2
                    nc.vector.tensor_tensor(
                        ob[:, 0:HALF], t1s[:], t2[:, 0:HALF], op=AOT.subtract
                    )
                    # mirror content: t1 + t2
                    if i < 2:
                        mc = mirpool.tile([128, N], F32R, tag=f"mc{i}")
                        nc.vector.tensor_tensor(
                            mc[:, 0:HALF], t1s[:], t2[:, 0:HALF], op=AOT.add
                        )
                        dsrc = mc
                        mirs[i] = mc
                    else:
                        dsrc = t1pool.tile([128, HALF], F32, tag="dtmp")
                        nc.vector.tensor_tensor(
                            dsrc[:, 0:HALF], t1s[:], t2[:, 0:HALF], op=AOT.add
                        )
                    # flipped column halves
                    nc.scalar.copy(ob[:, HALF : N], dsrc[:][:, FLW:0:-1])
                    if i < 2:
                        nc.scalar.copy(mc[:, HALF : N], ob[:][:, FLW:0:-1])
                    # row-0 patches: mirror tiles take the next block's row 0
                    if i == 1:
                        nc.vector.tensor_tensor(
                            mirs[0][0:1, 0:HALF], t1s[0:1, :], t2[0:1, 0:HALF],
                            op=AOT.add,
                        )
                        nc.scalar.copy(
                            mirs[0][0:1, HALF : N], ob[:][0:1, FLW:0:-1]
                        )
                    elif i == 2:
                        nc.vector.tensor_tensor(
                            mirs[1][0:1, 0:HALF], t1s[0:1, :], t2[0:1, 0:HALF],
                            op=AOT.add,
                        )
                        nc.scalar.copy(
                            mirs[1][0:1, HALF : N], ob[:][0:1, FLW:0:-1]
                        )
                    dma_engines[i % 2].dma_start(
                        out=out4[:][b][128 * i : 128 * (i + 1), :], in_=ob[:]
                    )
                    # last build chunk of the next sample goes here so the
                    # DVE stream reaches stage-2 assembly ops promptly
                    if b + 1 < BPC and i == 0:
                        emit_build_chunk(b + 1, 5)

    nc.compile()
    return nc


def _get_nc():
    global _NC
    if _NC is None:
        _NC = _build()
    return _NC


def _host_tables():
    a = np.arange(N, dtype=np.int64)
    ang = (2.0 * np.pi / N) * ((a[:, None] * a[None, :]) % N)
    sa = np.sqrt(ALPHA)
    ctv = (np.cos(ang) / N * sa).astype(np.float16)
    stv = (np.sin(ang) / N * sa).astype(np.float16)
    return ctv, stv


def _host_entry_tables(list_indices, coeff):
    """Bucket each expert's (u, v, coeff) entries by (v-chunk, u-half), pad
    buckets to PAD, and interleave the three tables partition-major so one
    row-granular gather of 72 floats per partition fetches everything."""
    li = list_indices.astype(np.int64)
    uu = li // N
    vv = li % N
    u3 = np.zeros((E, EROW), np.float32)
    vm3 = np.full((E, EROW), -9.0, np.float32)
    cv3 = np.zeros((E, EROW), np.float32)
    for e in range(E):
        for j in range(NBLK):
            selj = vv[e] // 128 == j
            base = BROW * j
            for half in range(2):
                u0 = UW * half
                sel = np.where(selj & (uu[e] >= u0) & (uu[e] < u0 + UW))[0]
                cnt = len(sel)
                assert cnt <= PAD, f"bucket overflow: e{e} j{j} h{half}: {cnt}"
                u3[e, base : base + cnt] = uu[e, sel]
                vm3[e, base : base + cnt] = vv[e, sel] - 128 * j
                cv3[e, base : base + cnt] = coeff[e, sel]
                base += PAD
    # partition-major interleave: uvc[e, p*72 + 3g + t] = tab_t[e, 128*g + p]
    s = np.arange(EROW)
    g, p = s // 128, s % 128
    idx = p * TCOLS + 3 * g
    uvcv = np.zeros((E, 3 * EROW), np.float32)
    uvcv[:, idx] = u3[:, s]
    uvcv[:, idx + 1] = vm3[:, s]
    uvcv[:, idx + 2] = cv3[:, s]
    return uvcv


def kernel(cls_token, W_router, b_router, coeff, list_indices):
    global LAST_RESULT
    cls_token = np.asarray(cls_token)
    W_router = np.asarray(W_router)
    b_router = np.asarray(b_router)
    coeff = np.asarray(coeff)
    list_indices = np.asarray(list_indices)
    assert cls_token.shape == (B, N) and coeff.shape == (E, NF)
    nc = _get_nc()
    ctv, stv = _host_tables()
    uvcv = _host_entry_tables(list_indices, coeff)
    basesv = np.tile(
        (np.arange(E, dtype=np.float32) * (3 * EROW)).reshape(E, 1), (2, 1)
    )
    jmv = np.zeros((128, 128), np.float32)
    for m_ in range(128):
        jmv[(128 - m_) % 128, m_] = 1.0
    wrr = np.ascontiguousarray(W_router.T, dtype=np.float32)
    brr = np.ascontiguousarray(b_router, dtype=np.float32)
    in_maps = []
    for c in range(NCORES):
        in_maps.append(
            {
                "cls4": np.ascontiguousarray(
                    cls_token[BPC * c : BPC * (c + 1)].T, dtype=np.float32
                ),
                "wr": wrr,
                "br": brr,
                "uvc": uvcv,
                "bases": basesv,
                "jm": jmv,
                "ct": ctv,
                "st": stv,
            }
        )
    res = run_bass_kernel_spmd(
        nc, in_maps, core_ids=list(range(NCORES)), trace=KERNEL_TRACE
    )
    LAST_RESULT = res
    out = np.concatenate([res.results[c]["out4"] for c in range(NCORES)], axis=0)
    return out


# revision 25
# speedup vs baseline: 1.0300x; 1.0192x over previous
# MoE routing + sparse-frequency inverse FFT2 kernel for Trainium2 (8 NeuronCores).
#
# Math: out_b = ALPHA * Re(ifft2(mask_b)) where mask_b has 4096 nonzero
# frequencies (top-2 experts x 2048 each).  With the symmetric real DFT basis
#   C[x,u] = cos(2*pi*x*u/768)/768,  S[x,u] = sin(2*pi*x*u/768)/768
# the dense iFFT2 factorizes per sample into  out = a*(C M C - S M S)  and the
# four-fold symmetry
#   out[x, y]     = t1 - t2          t1 = a*(C M C)[x, y<386]
#   out[x, N-y]   = t1 + t2          t2 = a*(S M S)[x, y<386]
#   out[N-x, y]   = t1 + t2
#   out[N-x, N-y] = t1 - t2
# lets stage 2 compute only 386 of 768 columns; full rows are assembled with
# negative-stride copies and rows 512..767 with an anti-identity row-mirror
# matmul.  sqrt(ALPHA) is folded into the C/S tables on the host.
# Device work per core (4 samples): router GEMM, top-2 selection and weights,
# per-expert entry gather (row-granular indirect DMA of one interleaved
# (u, v, coeff) table), sparse->dense mask build via iota/compare one-hots
# placed with PE matmuls (2 x 384-wide u-buckets per v-chunk, pad 256), then
# stage-1 P=M@C / Q=M@S at 386 columns and quadrant-assembled stage 2.
#
# Element-granular DMA scatter is avoided on purpose: TRN2's indirect DMA is
# row-granular (one offset per partition, contiguous run per partition), so
# the mask is built from gathered (u, v, val) entry groups instead.

import sys

sys.path.insert(0, "/opt/trn_rl_repo")

import numpy as np

import concourse.bacc as bacc
import concourse.mybir as mybir
import concourse.tile as tile
from concourse.bass import IndirectOffsetOnAxis
from concourse.bass_utils import run_bass_kernel_spmd
from concourse.masks import make_identity

N = 768
E = 64
NF = 2048
B = 32
NCORES = 8
BPC = B // NCORES          # samples per core
NBLK = 6                   # 768 / 128
ALPHA = 300.0
GRID = N * N
HALF = N // 2 + 2          # 386 computed stage-1/2 columns (even for f32r)
FLW = HALF - 4             # 382: width of the mirrored column range

# per-(expert, v-chunk, u-half) buckets: u in [0,384) and [384,768), each
# padded to 256 entries (exact max fill for the fixed input draw is 207).
UW = 384                   # u-bucket width (one psum bank at f32)
PAD = 256                  # entries per bucket -> 2 gather columns
BROW = 2 * PAD             # 512 entries per (expert, v-chunk)
EROW = NBLK * BROW         # 3072 entries per expert
COLS = EROW // 128         # 24 gather columns per expert
TCOLS = 3 * COLS           # 72: interleaved (u, vm, coeff) merged table

F32 = mybir.dt.float32
F32R = mybir.dt.float32r
F16 = mybir.dt.float16
I32 = mybir.dt.int32
AOT = mybir.AluOpType

KERNEL_TRACE = False       # test harness can flip this to profile
LAST_RESULT = None

_NC = None


def _build():
    nc = bacc.Bacc(trn_type="TRN2")

    cls4 = nc.dram_tensor("cls4", [N, BPC], F32, kind="ExternalInput")
    wr = nc.dram_tensor("wr", [N, E], F32, kind="ExternalInput")
    br = nc.dram_tensor("br", [E], F32, kind="ExternalInput")
    uvc = nc.dram_tensor("uvc", [E, 3 * EROW], F32, kind="ExternalInput")
    bases = nc.dram_tensor("bases", [2 * E, 1], F32, kind="ExternalInput")
    jm = nc.dram_tensor("jm", [128, 128], F32R, kind="ExternalInput")
    ct = nc.dram_tensor("ct", [N, N], F16, kind="ExternalInput")
    st = nc.dram_tensor("st", [N, N], F16, kind="ExternalInput")
    out4 = nc.dram_tensor("out4", [BPC, N, N], F32, kind="ExternalOutput")

    with tile.TileContext(nc) as tc:
        with (
            tc.tile_pool(name="const", bufs=1) as cpool,
            tc.tile_pool(name="tables", bufs=1) as tpool,
            tc.tile_pool(name="routing", bufs=1) as rpool,
            tc.tile_pool(name="gath", bufs=1) as gpool,
            tc.tile_pool(name="build", bufs=18) as bpool,
            tc.tile_pool(name="mt", bufs=2) as mtpool,
            tc.tile_pool(name="pq", bufs=2) as pqpool,
            tc.tile_pool(name="outp", bufs=4) as opool,
            tc.tile_pool(name="psH", bufs=6, space="PSUM") as psH,
            tc.tile_pool(name="psB", bufs=2, space="PSUM") as psB,
            tc.tile_pool(name="mir", bufs=2) as mirpool,
            tc.tile_pool(name="t1p", bufs=2) as t1pool,
        ):
            ident = cpool.tile([128, 128], F32)
            make_identity(nc, ident[:])
            ones1 = cpool.tile([1, 128], F32)
            nc.vector.memset(ones1[:], 1.0)
            ones14 = cpool.tile([1, BPC], F32)
            nc.vector.memset(ones14[:], 1.0)
            i768 = cpool.tile([128, N], I32)
            nc.gpsimd.iota(i768[:], pattern=[[1, N]], base=0, channel_multiplier=0)
            i768f = cpool.tile([128, N], F16)
            nc.vector.tensor_copy(i768f[:], i768[:])
            i128f = cpool.tile([128, 128], F16)
            nc.vector.tensor_copy(i128f[:], i768[:, 0:128])
            io72 = cpool.tile([128, 1], I32)
            nc.gpsimd.iota(io72[:], pattern=[[0, 1]], base=0, channel_multiplier=TCOLS)
            io72f = cpool.tile([128, 1], F32)
            nc.vector.tensor_copy(io72f[:], io72[:])

            br_sb = rpool.tile([1, E], F32)
            nc.sync.dma_start(out=br_sb[:], in_=br[None, :])
            bases_sb = rpool.tile([2 * E, 1], F32)
            nc.sync.dma_start(out=bases_sb[:], in_=bases[:])
            jJ = cpool.tile([128, 128], F32R)
            nc.sync.dma_start(out=jJ[:], in_=jm[:])

            # ---- router: logits = cls @ Wr.T + br (host pre-transposed) ----
            clsT = rpool.tile([128, NBLK * BPC], F32)
            wrT = rpool.tile([128, NBLK * E], F32)
            for j in range(NBLK):
                nc.sync.dma_start(
                    out=clsT[:, BPC * j : BPC * (j + 1)],
                    in_=cls4[128 * j : 128 * (j + 1), :],
                )
                nc.sync.dma_start(
                    out=wrT[:, E * j : E * (j + 1)],
                    in_=wr[128 * j : 128 * (j + 1), :],
                )
            lg_ps = psB.tile([BPC, E], F32, tag="small")
            for j in range(NBLK):
                nc.tensor.matmul(
                    lg_ps[:],
                    lhsT=clsT[:, BPC * j : BPC * (j + 1)],
                    rhs=wrT[:, E * j : E * (j + 1)],
                    start=(j == 0),
                    stop=False,
                )
            nc.tensor.matmul(
                lg_ps[:], lhsT=ones14[:], rhs=br_sb[:], start=False, stop=True
            )
            logits = rpool.tile([BPC, E], F32)
            nc.vector.tensor_copy(logits[:], lg_ps[:])

            # ---- top-2, renormalized weights, one-hot selectors ----
            max8 = rpool.tile([BPC, 8], F32)
            nc.vector.max(out=max8[:], in_=logits[:])
            l0 = max8[:, 0:1]
            l1 = max8[:, 1:2]
            d = rpool.tile([BPC, 1], F32)
            nc.vector.tensor_sub(d[:], l1, l0)  # l1 - l0
            dT_ps = psB.tile([1, BPC], F32, tag="small")
            nc.tensor.transpose(dT_ps[:], d[:], ident[0:BPC, 0:BPC])
            dT = rpool.tile([1, BPC], F32)
            nc.vector.tensor_copy(dT[:], dT_ps[:])
            w1T = rpool.tile([1, BPC], F32)
            nc.scalar.activation(w1T[:], dT[:], mybir.ActivationFunctionType.Sigmoid)
            w0T = rpool.tile([1, BPC], F32)
            nc.scalar.activation(
                w0T[:], dT[:], mybir.ActivationFunctionType.Sigmoid, scale=-1.0
            )
            oh1 = rpool.tile([BPC, E], F32)
            oh2 = rpool.tile([BPC, E], F32)
            nc.vector.tensor_scalar(oh1[:], logits[:], l0, None, op0=AOT.is_equal)
            nc.vector.tensor_scalar(oh2[:], logits[:], l1, None, op0=AOT.is_equal)
            selT = []
            for srcap in (oh1, oh2):
                sp = psB.tile([E, BPC], F32, tag="small")
                nc.tensor.transpose(sp[:], srcap[:], ident[0:BPC, 0:BPC])
                sbt = rpool.tile([E, BPC], F32, tag=f"selT{len(selT)}")
                nc.vector.tensor_copy(sbt[:], sp[:])
                selT.append(sbt)
            o1T, o2T = selT

            # scalar rows packed [eT0 | eT1 | w0 | w1], broadcast in one matmul
            stack = rpool.tile([1, 4 * BPC], F32)
            for si, oT in enumerate((o1T, o2T)):
                ep = psB.tile([1, BPC], F32, tag="small")
                nc.tensor.matmul(
                    ep[:], lhsT=bases_sb[0:E, :], rhs=oT[:], start=True, stop=True
                )
                nc.vector.tensor_copy(stack[:, BPC * si : BPC * (si + 1)], ep[:])
            nc.scalar.copy(stack[:, 2 * BPC : 3 * BPC], w0T[:])
            nc.scalar.copy(stack[:, 3 * BPC : 4 * BPC], w1T[:])
            bp = psB.tile([128, 4 * BPC], F32, tag="small")
            nc.tensor.matmul(bp[:], lhsT=ones1[:], rhs=stack[:], start=True, stop=True)
            bcT = rpool.tile([128, 4 * BPC], F32)
            nc.vector.tensor_copy(bcT[:], bp[:])
            ebc = [bcT[:, 0:BPC], bcT[:, BPC : 2 * BPC]]
            wbc = [bcT[:, 2 * BPC : 3 * BPC], bcT[:, 3 * BPC : 4 * BPC]]

            # ---- C/S table loads AFTER the routing-phase emission so the
            # small router DMAs aren't queued behind 4.7 MB on the sync FIFO
            ct_sb = tpool.tile([128, NBLK * N], F16, tag="ct")
            st_sb = tpool.tile([128, NBLK * N], F16, tag="st")
            for j in range(NBLK):
                nc.sync.dma_start(
                    out=ct_sb[:, N * j : N * (j + 1)],
                    in_=ct[128 * j : 128 * (j + 1), :],
                )
                nc.sync.dma_start(
                    out=st_sb[:, N * j : N * (j + 1)],
                    in_=st[128 * j : 128 * (j + 1), :],
                )

            dma_engines = (nc.sync, nc.scalar)

            # ---- gather ALL samples' interleaved (u, vm, coeff) runs upfront
            # so later samples' indirect dispatches aren't head-of-line
            # blocked on the gpsimd queue behind dependent work.
            off_tiles = {}
            for b in range(BPC):
                for slot in range(2):
                    offf = gpool.tile(
                        [128, 1], F32, tag=f"offf{b}_{slot}", name=f"offf{b}_{slot}"
                    )
                    nc.vector.tensor_add(offf[:], ebc[slot][:, b : b + 1], io72f[:])
                    offs = gpool.tile(
                        [128, 1], I32, tag=f"offs{b}_{slot}", name=f"offs{b}_{slot}"
                    )
                    nc.vector.tensor_copy(offs[:], offf[:])
                    off_tiles[(b, slot)] = offs
            allg = []
            for b in range(BPC):
                gms = []
                for slot in range(2):
                    gm = gpool.tile([128, TCOLS], F32, tag=f"gm{b}_{slot}")
                    nc.gpsimd.indirect_dma_start(
                        out=gm[:],
                        out_offset=None,
                        in_=uvc[:],
                        in_offset=IndirectOffsetOnAxis(
                            ap=off_tiles[(b, slot)][:], axis=1
                        ),
                    )
                    gms.append(gm)
                allg.append((gms, b))

            # ---- build MT (transposed mask) chunk by chunk on PE ----
            # per (j, u-half): one single-bank psum tile accumulating
            # 2 slots x 2 groups of one-hot placement matmuls.  Build of
            # sample b+1 is emitted interleaved into stage 1 of sample b so
            # the DVE one-hot production overlaps PE-dense stage matmuls.
            mt_tiles = {}

            gcw_tiles = {}

            def emit_build_chunk(b, j):
                gms, _ = allg[b]
                if j == 0:
                    mt_tiles[b] = mtpool.tile(
                        [128, NBLK * N], F16, tag="mt", name=f"mt{b}"
                    )
                    gcws = []
                    for slot in range(2):
                        gcw = gpool.tile(
                            [128, COLS], F32, tag=f"gcw{b}_{slot}",
                            name=f"gcw{b}_{slot}",
                        )
                        nc.vector.tensor_scalar(
                            gcw[:], gms[slot][:][:, 2:TCOLS:3],
                            wbc[slot][:, b : b + 1], None, op0=AOT.mult,
                        )
                        gcws.append(gcw)
                    gcw_tiles[b] = gcws
                mt_sb = mt_tiles[b]
                gcws = gcw_tiles[b]
                pss = [
                    psH.tile([128, 448], F32, tag="ps", name=f"bld{half}")
                    for half in range(2)
                ]
                for slot in range(2):
                    for g in range(2):
                        for half in range(2):
                            u0 = UW * half
                            c = 4 * j + 2 * half + g
                            voh = bpool.tile([128, 128], F16, tag="voh")
                            nc.vector.tensor_scalar(
                                voh[:], i128f[:],
                                gms[slot][:, 3 * c + 1 : 3 * c + 2],
                                None, op0=AOT.is_equal,
                            )
                            rhsb = bpool.tile([128, UW], F16, tag="rhsb")
                            nc.vector.tensor_scalar(
                                rhsb[:], i768f[:, u0 : u0 + UW],
                                gms[slot][:, 3 * c : 3 * c + 1],
                                gcws[slot][:, c : c + 1],
                                op0=AOT.is_equal, op1=AOT.mult,
                            )
                            nc.tensor.matmul(
                                pss[half][:, 0:UW],
                                lhsT=voh[:],
                                rhs=rhsb[:],
                                start=(slot == 0 and g == 0),
                                stop=(slot == 1 and g == 1),
                            )
                for half in range(2):
                    u0 = UW * half
                    nc.scalar.copy(
                        mt_sb[:, N * j + u0 : N * j + u0 + UW],
                        pss[half][:, 0:UW],
                    )

            for j in range(NBLK):
                emit_build_chunk(0, j)

            for b in range(BPC):
                mt_sb = mt_tiles[b]
                # ---- stage 1: P = M @ (sa*C), Q = M @ (sa*S), 386 cols;
                # weave next sample's build chunks between the i-blocks.
                p_sb = pqpool.tile([128, NBLK * HALF], F16, tag="p")
                q_sb = pqpool.tile([128, NBLK * HALF], F16, tag="q")
                for i in range(NBLK):
                    pps = psH.tile([128, 448], F32, tag="ps")
                    qps = psH.tile([128, 448], F32, tag="ps")
                    for k in range(NBLK):
                        lhs = mt_sb[:, N * k + 128 * i : N * k + 128 * (i + 1)]
                        nc.tensor.matmul(
                            pps[:, 0:HALF], lhsT=lhs,
                            rhs=ct_sb[:, N * k : N * k + HALF],
                            start=(k == 0), stop=(k == NBLK - 1),
                        )
                        nc.tensor.matmul(
                            qps[:, 0:HALF], lhsT=lhs,
                            rhs=st_sb[:, N * k : N * k + HALF],
                            start=(k == 0), stop=(k == NBLK - 1),
                        )
                    nc.scalar.copy(p_sb[:, HALF * i : HALF * (i + 1)], pps[:, 0:HALF])
                    nc.scalar.copy(q_sb[:, HALF * i : HALF * (i + 1)], qps[:, 0:HALF])
                    if b + 1 < BPC and i < 5:
                        emit_build_chunk(b + 1, i)

                # ---- stage 2 (quadrant): t1 = a*CMC, t2 = a*SMS, cols < 386;
                # out rows 0..511 direct, 512..767 via row-mirror matmul.
                def emit_mir(di, mc, dmae):
                    d0 = psH.tile([128, 448], F32, tag="ps")
                    d1 = psH.tile([128, 448], F32, tag="ps")
                    nc.tensor.matmul(
                        d0[:, 0:448], lhsT=jJ[:], rhs=mc[:, 0:448],
                        start=True, stop=True,
                    )
                    nc.tensor.matmul(
                        d1[:, 0:320], lhsT=jJ[:], rhs=mc[:, 448:768],
                        start=True, stop=True,
                    )
                    ob = opool.tile([128, N], F32, tag="ob")
                    nc.scalar.copy(ob[:, 0:448], d0[:, 0:448])
                    nc.scalar.copy(ob[:, 448:768], d1[:, 0:320])
                    dmae.dma_start(
                        out=out4[:][b][128 * (4 + di) : 128 * (5 + di), :], in_=ob[:]
                    )

                mirs = {}
                for i in range(4):
                    t1 = psH.tile([128, 448], F32, tag="ps")
                    t2 = psH.tile([128, 448], F32, tag="ps")
                    # mirror blocks are emitted one i-block late so their PE
                    # matmuls queue behind this block's chains and never wait
                    # on the previous block's assembly ops
                    pend_mir = None
                    if i == 2:
                        pend_mir = (1, mirs[0])
                    elif i == 3:
                        pend_mir = (0, mirs[1])
                    for k in range(NBLK):
                        nc.tensor.matmul(
                            t1[:, 0:HALF],
                            lhsT=ct_sb[:, N * k + 128 * i : N * k + 128 * (i + 1)],
                            rhs=p_sb[:, HALF * k : HALF * (k + 1)],
                            start=(k == 0), stop=(k == NBLK - 1),
                        )
                        nc.tensor.matmul(
                            t2[:, 0:HALF],
                            lhsT=st_sb[:, N * k + 128 * i : N * k + 128 * (i + 1)],
                            rhs=q_sb[:, HALF * k : HALF * (k + 1)],
                            start=(k == 0), stop=(k == NBLK - 1),
                        )
                    if pend_mir is not None:
                        emit_mir(pend_mir[0], pend_mir[1], dma_engines[i % 2])
                    t1s = t1pool.tile([128, HALF], F32, tag="t1s")
                    nc.scalar.copy(t1s[:], t1[:, 0:HALF])
                    ob = opool.tile([128, N], F32, tag="ob")
                    # direct region: out = t1 - t2
                    nc.vector.tensor_tensor(
                        ob[:, 0:HALF], t1s[:], t2[:, 0:HALF], op=AOT.subtract
                    )
                    # mirror content: t1 + t2
                    if i < 2:
                        mc = mirpool.tile([128, N], F32R, tag=f"mc{i}")
                        nc.vector.tensor_tensor(
                            mc[:, 0:HALF], t1s[:], t2[:, 0:HALF], op=AOT.add
                        )
                        dsrc = mc
                        mirs[i] = mc
                    else:
                        dsrc = t1pool.tile([128, HALF], F32, tag="dtmp")
                        nc.vector.tensor_tensor(
                            dsrc[:, 0:HALF], t1s[:], t2[:, 0:HALF], op=AOT.add
                        )
                    # flipped column halves
                    nc.scalar.copy(ob[:, HALF : N], dsrc[:][:, FLW:0:-1])
                    if i < 2:
                        nc.scalar.copy(mc[:, HALF : N], ob[:][:, FLW:0:-1])
                    # row-0 patches: mirror tiles take the next block's row 0
                    if i == 1:
                        nc.vector.tensor_tensor(
                            mirs[0][0:1, 0:HALF], t1s[0:1, :], t2[0:1, 0:HALF],
                            op=AOT.add,
                        )
                        nc.scalar.copy(
                            mirs[0][0:1, HALF : N], ob[:][0:1, FLW:0:-1]
                        )
                    elif i == 2:
                        nc.vector.tensor_tensor(
                            mirs[1][0:1, 0:HALF], t1s[0:1, :], t2[0:1, 0:HALF],
                            op=AOT.add,
                        )
                        nc.scalar.copy(
                            mirs[1][0:1, HALF : N], ob[:][0:1, FLW:0:-1]
                        )
                    dma_engines[i % 2].dma_start(
                        out=out4[:][b][128 * i : 128 * (i + 1), :], in_=ob[:]
                    )
                    # last build chunk of the next sample goes here so the
                    # DVE stream reaches stage-2 assembly ops promptly
                    if b + 1 < BPC and i == 0:
                        emit_build_chunk(b + 1, 5)

    nc.compile()
    return nc


def _get_nc():
    global _NC
    if _NC is None:
        _NC = _build()
    return _NC


def _host_tables():
    a = np.arange(N, dtype=np.int64)
    ang = (2.0 * np.pi / N) * ((a[:, None] * a[None, :]) % N)
    sa = np.sqrt(ALPHA)
    ctv = (np.cos(ang) / N * sa).astype(np.float16)
    stv = (np.sin(ang) / N * sa).astype(np.float16)
    return ctv, stv


def _host_entry_tables(list_indices, coeff):
    """Bucket each expert's (u, v, coeff) entries by (v-chunk, u-half), pad
    buckets to PAD, and interleave the three tables partition-major so one
    row-granular gather of 72 floats per partition fetches everything."""
    li = list_indices.astype(np.int64)
    uu = li // N
    vv = li % N
    u3 = np.zeros((E, EROW), np.float32)
    vm3 = np.full((E, EROW), -9.0, np.float32)
    cv3 = np.zeros((E, EROW), np.float32)
    for e in range(E):
        for j in range(NBLK):
            selj = vv[e] // 128 == j
            base = BROW * j
            for half in range(2):
                u0 = UW * half
                sel = np.where(selj & (uu[e] >= u0) & (uu[e] < u0 + UW))[0]
                cnt = len(sel)
                assert cnt <= PAD, f"bucket overflow: e{e} j{j} h{half}: {cnt}"
                u3[e, base : base + cnt] = uu[e, sel]
                vm3[e, base : base + cnt] = vv[e, sel] - 128 * j
                cv3[e, base : base + cnt] = coeff[e, sel]
                base += PAD
    # partition-major interleave: uvc[e, p*72 + 3g + t] = tab_t[e, 128*g + p]
    s = np.arange(EROW)
    g, p = s // 128, s % 128
    idx = p * TCOLS + 3 * g
    uvcv = np.zeros((E, 3 * EROW), np.float32)
    uvcv[:, idx] = u3[:, s]
    uvcv[:, idx + 1] = vm3[:, s]
    uvcv[:, idx + 2] = cv3[:, s]
    return uvcv


def kernel(cls_token, W_router, b_router, coeff, list_indices):
    global LAST_RESULT
    cls_token = np.asarray(cls_token)
    W_router = np.asarray(W_router)
    b_router = np.asarray(b_router)
    coeff = np.asarray(coeff)
    list_indices = np.asarray(list_indices)
    assert cls_token.shape == (B, N) and coeff.shape == (E, NF)
    nc = _get_nc()
    ctv, stv = _host_tables()
    uvcv = _host_entry_tables(list_indices, coeff)
    basesv = np.tile(
        (np.arange(E, dtype=np.float32) * (3 * EROW)).reshape(E, 1), (2, 1)
    )
    jmv = np.zeros((128, 128), np.float32)
    for m_ in range(128):
        jmv[(128 - m_) % 128, m_] = 1.0
    wrr = np.ascontiguousarray(W_router.T, dtype=np.float32)
    brr = np.ascontiguousarray(b_router, dtype=np.float32)
    in_maps = []
    for c in range(NCORES):
        in_maps.append(
            {
                "cls4": np.ascontiguousarray(
                    cls_token[BPC * c : BPC * (c + 1)].T, dtype=np.float32
                ),
                "wr": wrr,
                "br": brr,
                "uvc": uvcv,
                "bases": basesv,
                "jm": jmv,
                "ct": ctv,
                "st": stv,
            }
        )
    res = run_bass_kernel_spmd(
        nc, in_maps, core_ids=list(range(NCORES)), trace=KERNEL_TRACE
    )
    LAST_RESULT = res
    out = np.concatenate([res.results[c]["out4"] for c in range(NCORES)], axis=0)
    return out
